# revision 26
# baseline (speedup 1.0000x reference)
"""AdaptiveMLP Trainium2 kernel (8-core data parallel).

Math per layer: y[b,o] = sum_{n,i} co[b,n]*x[b,i]*W[n,i,o] + sum_n co[b,n]*b[n,o]

Decomposition per core (B=8192 samples, feature-major / transposed chain):
  - u0co^T [40, B]: rows (n,i) n*3+i = co_n*x_i (30 rows), rows 30+n = co_n.
    Built batch-major with one broadcast-AP tensor_tensor op, then PE-transposed.
  - L0: z1^T = W0flat^T @ u0co^T  (W0flat rows 30..39 carry b0) -> relu -> x1aug^T [65,B]
    (row 64 = ones).
  - L1 (per group-pair p, per 512-col chunk c):
      t^T   = [W1aug_n | W1aug_m]^T @ x1aug^T  -> psum [128,512] -> bf16 sbuf
      cb    = S64_p^T @ co^T (selector broadcast of co rows) -> psum -> bf16 sbuf
      m     = t * cb  (DVE bf16)
      z2^T += R2^T @ m (PSUM-accumulated selector reduce over the pair's 2 groups)
    relu -> x2aug^T.
  - L2: 4 chunks partition-stacked: t2 [4*32,512], cb3 (selector with per-chunk
    columns), m2, R3 reduce -> y^T -> PE transpose back to batch-major -> DMA out.

All matmul inputs bf16 (PE 1 cyc/row), accumulation fp32 in PSUM.
"""
import sys

sys.path.insert(0, "/opt/trn_rl_repo")

import numpy as np

import concourse.bacc as bacc
import concourse.bass as bass
import concourse.mybir as mybir
import concourse.tile as tile
from concourse.bass_utils import run_bass_kernel_spmd
from concourse.tile_rust import add_dep_helper

N_CORES = 8
B = 65536
G = 10
CI, H, CO = 3, 64, 3
B_LOC = B // N_CORES

F32 = mybir.dt.float32
BF16 = mybir.dt.bfloat16


def host_constants(W0, W1, W2, b0, b1, b2):
    """Pack all constant matrices into two blobs (fp32; cast to bf16 on load).

    blob42 [42, 832]: W0flat[0:64] | S64[64:704] | B1sel[704:768] | B2sel[768:800] | S3[800:832]
    blob128 [128, 524]: W1s[0:320] | W2lo[320:352] | W2hi[352:384] | R3[384:396] | ident[396:524]
    """
    blob42 = np.zeros((42, 832), np.float32)
    W0flat = blob42[:, 0:64]
    S64 = blob42[:, 64:704]
    B1sel = blob42[:, 704:768]
    B2sel = blob42[:, 768:800]
    S3 = blob42[:, 800:832]
    for n in range(G):
        for i in range(CI):
            W0flat[n * 3 + i] = W0[n, i]
        W0flat[32 + n] = b0[n]
        B1sel[32 + n] = b1[n]
        for o in range(CO):
            B2sel[32 + n, n * 3 + o] = b2[n, o]
            S3[32 + n, n * 3 + o] = 1.0
    for p in range(5):
        S64[32 + 2 * p, p * 128:p * 128 + H] = 1.0
        S64[32 + 2 * p + 1, p * 128 + H:(p + 1) * 128] = 1.0
    blob128 = np.zeros((128, 524), np.float32)
    W1s = blob128[:, 0:320]
    W2lo = blob128[0:64, 320:352]
    W2hi = blob128[64:128, 352:384]
    R3 = blob128[:, 384:396]
    ident = blob128[:, 396:524]
    for p in range(5):
        W1s[:H, p * H:(p + 1) * H] = W1[2 * p]
        W1s[H:, p * H:(p + 1) * H] = W1[2 * p + 1]
    for n in range(G):
        for o in range(CO):
            W2lo[:, n * 3 + o] = W2[n, :, o]
            W2hi[:, n * 3 + o] = W2[n, :, o]
    for c in range(4):
        for n in range(G):
            for o in range(CO):
                R3[32 * c + n * 3 + o, c * 3 + o] = 1.0
    np.fill_diagonal(ident, 1.0)
    return dict(blob42=blob42, blob128=blob128)


def make_reps(x_loc, co_loc, b_loc=B_LOC):
    """Host-side zero-flop replication: feature-major row-replicated x and co
    in u0coT row layout (rows n*3+i -> x_i / co_n; rows 32+n -> 1 / co_n)."""
    import ml_dtypes
    S = b_loc // 128
    xT = x_loc.reshape(128, S, CI).transpose(2, 1, 0).reshape(CI, b_loc)
    coT = co_loc.reshape(128, S, G).transpose(2, 1, 0).reshape(G, b_loc)
    xrep = np.zeros((42, b_loc), np.float32)
    corep = np.zeros((42, b_loc), np.float32)
    for n in range(G):
        for i in range(CI):
            xrep[n * 3 + i] = xT[i]
            corep[n * 3 + i] = coT[n]
        xrep[32 + n] = 1.0
        corep[32 + n] = coT[n]
    return xrep.astype(ml_dtypes.bfloat16), corep.astype(ml_dtypes.bfloat16)


def make_cbrep(co_loc, b_loc=B_LOC):
    """Host-side zero-flop layout prep: replicate co rows into the broadcast
    layout the kernel's multiply expects (bf16, u0coT column order
    col = s*128 + p <-> sample b = p*S + s)."""
    import ml_dtypes
    S = b_loc // 128
    arr = co_loc.astype(ml_dtypes.bfloat16)          # [b_loc, 10]
    coT = arr.reshape(128, S, G).transpose(2, 1, 0).reshape(G, b_loc)
    cb = np.empty((5, 128, b_loc), dtype=ml_dtypes.bfloat16)
    for p in range(5):
        cb[p, :64] = coT[2 * p]
        cb[p, 64:] = coT[2 * p + 1]
    return cb


def build(nc, b_loc=B_LOC):
    TILES = b_loc // 128       # 128-sample tiles
    CHUNKS = b_loc // 512      # 512-col chunks
    GROUPS = CHUNKS // 4       # L2 4-chunk groups
    assert CHUNKS % 4 == 0

    xr_d = nc.declare_dram_parameter("xrep", [42, b_loc], BF16, isOutput=False)
    cor_d = nc.declare_dram_parameter("corep", [42, b_loc], BF16, isOutput=False)
    b42_d = nc.declare_dram_parameter("blob42", [42, 832], F32, isOutput=False)
    b128_d = nc.declare_dram_parameter("blob128", [128, 524], F32, isOutput=False)
    cb_d = nc.declare_dram_parameter("cbrep", [5, 128, b_loc], BF16, isOutput=False)
    out_d = nc.declare_dram_parameter("out", [b_loc, CO], F32, isOutput=True)

    with tile.TileContext(nc) as tc:
        with (
            tc.tile_pool(name="consts", bufs=1) as consts,
            tc.tile_pool(name="chain", bufs=1) as chain,
            tc.tile_pool(name="stream", bufs=8) as stream,
            tc.tile_pool(name="stream2", bufs=3) as stream2,
            tc.tile_pool(name="psT", bufs=1, space="PSUM") as psT,
            tc.tile_pool(name="psA", bufs=5, space="PSUM") as psA,
            tc.tile_pool(name="psB", bufs=2, space="PSUM") as psB,
        ):
            # ---- small loads first (sync ring): xrep/corep/blobs ----
            S = b_loc // 128
            b42_f = consts.tile([42, 832], F32)
            nc.sync.dma_start(b42_f[:], b42_d[:])
            b128_f = consts.tile([128, 524], F32)
            nc.sync.dma_start(b128_f[:], b128_d[:])
            xrep = chain.tile([42, b_loc], BF16, tag="bigA")
            corep = chain.tile([42, b_loc], BF16, tag="bigB")
            hb = b_loc // 2
            nc.sync.dma_start(xrep[:, 0:hb], xr_d[:, 0:hb])
            nc.sync.dma_start(corep[:, 0:hb], cor_d[:, 0:hb])
            nc.sync.dma_start(xrep[:, hb:], xr_d[:, hb:])
            corep_dma = nc.sync.dma_start(corep[:, hb:], cor_d[:, hb:])
            b42 = consts.tile([42, 832], BF16)
            nc.vector.tensor_copy(b42[:], b42_f[:])
            b128 = consts.tile([128, 524], BF16)
            nc.vector.tensor_copy(b128[:], b128_f[:])
            W0f = b42[:, 0:64]
            S64 = b42[:, 64:704]
            B1 = b42[:, 704:768]
            B2 = b42[:, 768:800]
            S3 = b42[:, 800:832]
            W1s = b128[:, 0:320]
            W2lo = b128[:, 320:352]
            W2hi = b128[:, 352:384]
            R3 = b128[:, 384:396]
            ident_b = b128[:, 396:524]
            # ---- cbrep, column-sliced; ordered after corep so the critical
            # small loads get the DMA bandwidth first ----
            DT = 2048
            cbs = []
            for p in range(5):
                cb_t = chain.tile([128, b_loc], BF16, tag=f"cb{p}")
                cbs.append(cb_t)
            for c4 in range(b_loc // DT):
                sl = slice(c4 * DT, (c4 + 1) * DT)
                for p in range(5):
                    cb_dma = nc.scalar.dma_start(cbs[p][:, sl], cb_d[p, :, sl])
                    add_dep_helper(cb_dma.ins, corep_dma.ins,
                                   reason="cbrep after critical input loads")

            # ---- u0coT = xrep * corep (feature-major, split for overlap) ----
            u0coT = chain.tile([42, b_loc], BF16)
            for qq in range(4):
                sl = slice(qq * (b_loc // 4), (qq + 1) * (b_loc // 4))
                nc.vector.tensor_tensor(
                    out=u0coT[:, sl], in0=xrep[:, sl], in1=corep[:, sl],
                    op=mybir.AluOpType.mult,
                )

            # ---- L0: z1T = W0f^T @ u0coT ; relu -> x1dup rows 0:64 ----
            x1dup = chain.tile([128, b_loc], BF16)
            x2dual = chain.tile([128, b_loc // 2], BF16)
            for c in range(CHUNKS):
                pz = psA.tile([H, 512], F32, tag="z")
                nc.tensor.matmul(pz[:], W0f[:], u0coT[:, c * 512:(c + 1) * 512])
                nc.scalar.activation(
                    x1dup[:H, c * 512:(c + 1) * 512], pz[:],
                    mybir.ActivationFunctionType.Relu,
                )
            # duplicate rows 0:64 -> 64:128 (sbuf->sbuf DMA partition move)
            for qq in range(4):
                sl = slice(qq * (b_loc // 4), (qq + 1) * (b_loc // 4))
                nc.sync.dma_start(x1dup[H:, sl], x1dup[:H, sl])

            # ---- L1: z2 = sum_p W1s_p^T @ (x1dup * cb_p) + B1^T @ u0coT ----
            # col-tiled: chunk pair (2d, 2d+1) -> one [128, 512] psum bank
            for c4 in range(CHUNKS // 4):
                xps = []
                for p in range(5):
                    xp = stream.tile([128, DT], BF16, tag="xp")
                    nc.vector.tensor_tensor(
                        out=xp[:], in0=x1dup[:, c4 * DT:(c4 + 1) * DT],
                        in1=cbs[p][:, c4 * DT:(c4 + 1) * DT],
                        op=mybir.AluOpType.mult,
                    )
                    xps.append(xp)
                for dd in range(2):  # two chunk-pairs per TT block
                    pz2 = psA.tile([128, 512], F32, tag="z")
                    for p in range(5):
                        for h in range(2):
                            cc = 2 * dd + h
                            nc.tensor.matmul(
                                pz2[64 * h:64 * h + 64, :],
                                W1s[:, p * H:(p + 1) * H],
                                xps[p][:, cc * 512:(cc + 1) * 512],
                                tile_position=(0, 64 * h),
                                start=(p == 0), stop=False,
                                skip_group_check=True,
                            )
                    for h in range(2):
                        c = 4 * c4 + 2 * dd + h
                        nc.tensor.matmul(
                            pz2[64 * h:64 * h + 64, :], B1[:],
                            u0coT[:, c * 512:(c + 1) * 512],
                            tile_position=(0, 64 * h),
                            start=False, stop=True,
                            skip_group_check=True,
                        )
                    d_abs = 2 * c4 + dd
                    nc.scalar.activation(
                        x2dual[:, d_abs * 512:(d_abs + 1) * 512], pz2[:],
                        mybir.ActivationFunctionType.Relu,
                    )

            # ---- L2 (4-chunk groups, partition-stacked; t-scheme, 30-wide) ----
            y_bm = chain.tile([128, S * CO], F32)
            for g in range(GROUPS):
                pt2 = psB.tile([128, 512], F32, tag="cbps")
                pcb3 = psB.tile([128, 512], F32, tag="cbps")
                for q in range(4):
                    c = 4 * g + q
                    d_abs, h = divmod(c, 2)
                    W2v = W2lo if h == 0 else W2hi
                    nc.tensor.matmul(
                        pt2[32 * q:32 * q + 32, :], W2v[:],
                        x2dual[:, d_abs * 512:(d_abs + 1) * 512],
                        tile_position=(0, 32 * q),
                        start=True, stop=False,
                        skip_group_check=True,
                    )
                for q in range(4):
                    c = 4 * g + q
                    nc.tensor.matmul(
                        pt2[32 * q:32 * q + 32, :], B2[:],
                        u0coT[:, c * 512:(c + 1) * 512],
                        tile_position=(0, 32 * q),
                        start=False, stop=True,
                        skip_group_check=True,
                    )
                for q in range(4):
                    c = 4 * g + q
                    nc.tensor.matmul(
                        pcb3[32 * q:32 * q + 32, :], S3[:],
                        u0coT[:, c * 512:(c + 1) * 512],
                        tile_position=(0, 32 * q),
                    )
                t2_sb = stream2.tile([128, 512], BF16, tag="t2_sb")
                nc.vector.tensor_copy(t2_sb[:], pt2[:])
                cb3_sb = stream2.tile([128, 512], BF16, tag="cb3_sb")
                nc.scalar.activation(
                    cb3_sb[:], pcb3[:], mybir.ActivationFunctionType.Copy
                )
                m2_sb = stream2.tile([128, 512], BF16, tag="m2_sb")
                nc.vector.tensor_tensor(
                    out=m2_sb[:], in0=t2_sb[:], in1=cb3_sb[:],
                    op=mybir.AluOpType.mult,
                )
                pyT = psA.tile([12, 512], F32, tag="z")
                nc.tensor.matmul(pyT[:], R3[:], m2_sb[:])
                yT_sb = stream2.tile([12, 512], BF16, tag="yT_sb")
                nc.vector.tensor_copy(yT_sb[:], pyT[:])
                for j in range(4):
                    pyb = psT.tile([128, 12], BF16, tag="tp_in")
                    nc.tensor.transpose(
                        pyb[:], yT_sb[:, j * 128:(j + 1) * 128],
                        ident_b[:12, :12],
                    )
                    y5 = y_bm[:].rearrange(
                        "p (gg q j o) -> p gg q j o", q=4, j=4, o=CO
                    )
                    nc.vector.tensor_copy(
                        y5[:, g, :, j, :],
                        pyb[:].rearrange("p (q o) -> p q o", o=CO),
                    )

            nc.sync.dma_start(
                out_d[:].rearrange("(p s) o -> p (s o)", p=128), y_bm[:]
            )
    nc.compile()
    return nc


_NC_CACHE = {}


def get_nc(b_loc=B_LOC):
    if b_loc not in _NC_CACHE:
        nc = bacc.Bacc(None, target_bir_lowering=False)
        _NC_CACHE[b_loc] = build(nc, b_loc)
    return _NC_CACHE[b_loc]


def kernel(input, co_mat, W0, W1, W2, b0, b1, b2, _trace=False):
    input = np.asarray(input, np.float32)
    co_mat = np.asarray(co_mat, np.float32)
    consts = host_constants(
        np.asarray(W0, np.float32), np.asarray(W1, np.float32),
        np.asarray(W2, np.float32), np.asarray(b0, np.float32),
        np.asarray(b1, np.float32), np.asarray(b2, np.float32),
    )
    nc = get_nc()
    in_maps = []
    for k in range(N_CORES):
        sl = slice(k * B_LOC, (k + 1) * B_LOC)
        xr, cr = make_reps(input[sl], co_mat[sl])
        m = {"xrep": xr, "corep": cr, "cbrep": make_cbrep(co_mat[sl])}
        m.update(consts)
        in_maps.append(m)
    res = run_bass_kernel_spmd(
        nc, in_maps, core_ids=list(range(N_CORES)), trace=_trace
    )
    out = np.concatenate([res.results[k]["out"] for k in range(N_CORES)], axis=0)
    if _trace:
        kernel.last_exec_time_ns = res.exec_time_ns
    return out


kernel.last_exec_time_ns = None


# revision 28
# speedup vs baseline: 1.3132x; 1.3132x over previous
"""AdaptiveMLP Trainium2 kernel (8-core data parallel).

Math per layer: y[b,o] = sum_{n,i} co[b,n]*x[b,i]*W[n,i,o] + sum_n co[b,n]*b[n,o]

Decomposition per core (B=8192 samples, feature-major / transposed chain):
  - u0co^T [40, B]: rows (n,i) n*3+i = co_n*x_i (30 rows), rows 30+n = co_n.
    Built batch-major with one broadcast-AP tensor_tensor op, then PE-transposed.
  - L0: z1^T = W0flat^T @ u0co^T  (W0flat rows 30..39 carry b0) -> relu -> x1aug^T [65,B]
    (row 64 = ones).
  - L1 (per group-pair p, per 512-col chunk c):
      t^T   = [W1aug_n | W1aug_m]^T @ x1aug^T  -> psum [128,512] -> bf16 sbuf
      cb    = S64_p^T @ co^T (selector broadcast of co rows) -> psum -> bf16 sbuf
      m     = t * cb  (DVE bf16)
      z2^T += R2^T @ m (PSUM-accumulated selector reduce over the pair's 2 groups)
    relu -> x2aug^T.
  - L2: 4 chunks partition-stacked: t2 [4*32,512], cb3 (selector with per-chunk
    columns), m2, R3 reduce -> y^T -> PE transpose back to batch-major -> DMA out.

All matmul inputs bf16 (PE 1 cyc/row), accumulation fp32 in PSUM.
"""
import sys

sys.path.insert(0, "/opt/trn_rl_repo")

import numpy as np

import concourse.bacc as bacc
import concourse.bass as bass
import concourse.mybir as mybir
import concourse.tile as tile
from concourse.bass_utils import run_bass_kernel_spmd

N_CORES = 8
B = 65536
G = 10
CI, H, CO = 3, 64, 3
B_LOC = B // N_CORES

F32 = mybir.dt.float32
BF16 = mybir.dt.bfloat16


def host_constants(W0, W1, W2, b0, b1, b2):
    """Pack all constant matrices into two blobs (fp32; cast to bf16 on load).

    blob42 [42, 832]: W0flat[0:64] | S64[64:704] | B1sel[704:768] | B2sel[768:800] | S3[800:832]
    blob128 [128, 524]: W1s[0:320] | W2lo[320:352] | W2hi[352:384] | R3[384:396] | ident[396:524]
    """
    blob42 = np.zeros((42, 832), np.float32)
    W0flat = blob42[:, 0:64]
    S64 = blob42[:, 64:704]
    B1sel = blob42[:, 704:768]
    B2sel = blob42[:, 768:800]
    S3 = blob42[:, 800:832]
    for n in range(G):
        for i in range(CI):
            W0flat[n * 3 + i] = W0[n, i]
        W0flat[32 + n] = b0[n]
        B1sel[32 + n] = b1[n]
        for o in range(CO):
            B2sel[32 + n, n * 3 + o] = b2[n, o]
            S3[32 + n, n * 3 + o] = 1.0
    for p in range(5):
        S64[32 + 2 * p, p * 128:p * 128 + H] = 1.0
        S64[32 + 2 * p + 1, p * 128 + H:(p + 1) * 128] = 1.0
    blob128 = np.zeros((128, 524), np.float32)
    W1s = blob128[:, 0:320]
    W2lo = blob128[0:64, 320:352]
    W2hi = blob128[64:128, 352:384]
    R3 = blob128[:, 384:396]
    ident = blob128[:, 396:524]
    for p in range(5):
        W1s[:H, p * H:(p + 1) * H] = W1[2 * p]
        W1s[H:, p * H:(p + 1) * H] = W1[2 * p + 1]
    for n in range(G):
        for o in range(CO):
            W2lo[:, n * 3 + o] = W2[n, :, o]
            W2hi[:, n * 3 + o] = W2[n, :, o]
    for c in range(4):
        for n in range(G):
            for o in range(CO):
                R3[32 * c + n * 3 + o, c * 3 + o] = 1.0
    np.fill_diagonal(ident, 1.0)
    return dict(blob42=blob42, blob128=blob128)


def make_reps(x_loc, co_loc, b_loc=B_LOC):
    """Host-side zero-flop replication: feature-major row-replicated x and co
    in u0coT row layout (rows n*3+i -> x_i / co_n; rows 32+n -> 1 / co_n)."""
    import ml_dtypes
    S = b_loc // 128
    xT = x_loc.reshape(128, S, CI).transpose(2, 1, 0).reshape(CI, b_loc)
    coT = co_loc.reshape(128, S, G).transpose(2, 1, 0).reshape(G, b_loc)
    xrep = np.zeros((42, b_loc), np.float32)
    corep = np.zeros((42, b_loc), np.float32)
    for n in range(G):
        for i in range(CI):
            xrep[n * 3 + i] = xT[i]
            corep[n * 3 + i] = coT[n]
        xrep[32 + n] = 1.0
        corep[32 + n] = coT[n]
    return xrep.astype(ml_dtypes.bfloat16), corep.astype(ml_dtypes.bfloat16)


def make_cbrep(co_loc, b_loc=B_LOC):
    """Host-side zero-flop layout prep: replicate co rows into the broadcast
    layout the kernel's multiply expects (bf16, u0coT column order
    col = s*128 + p <-> sample b = p*S + s)."""
    import ml_dtypes
    S = b_loc // 128
    arr = co_loc.astype(ml_dtypes.bfloat16)          # [b_loc, 10]
    coT = arr.reshape(128, S, G).transpose(2, 1, 0).reshape(G, b_loc)
    cb = np.empty((5, 128, b_loc), dtype=ml_dtypes.bfloat16)
    for p in range(5):
        cb[p, :64] = coT[2 * p]
        cb[p, 64:] = coT[2 * p + 1]
    return cb


def build(nc, b_loc=B_LOC):
    TILES = b_loc // 128       # 128-sample tiles
    CHUNKS = b_loc // 512      # 512-col chunks
    GROUPS = CHUNKS // 4       # L2 4-chunk groups
    assert CHUNKS % 4 == 0

    xr_d = nc.declare_dram_parameter("xrep", [42, b_loc], BF16, isOutput=False)
    cor_d = nc.declare_dram_parameter("corep", [42, b_loc], BF16, isOutput=False)
    b42_d = nc.declare_dram_parameter("blob42", [42, 832], F32, isOutput=False)
    b128_d = nc.declare_dram_parameter("blob128", [128, 524], F32, isOutput=False)
    cb_d = nc.declare_dram_parameter("cbrep", [5, 128, b_loc], BF16, isOutput=False)
    out_d = nc.declare_dram_parameter("out", [b_loc, CO], F32, isOutput=True)

    with tile.TileContext(nc) as tc:
        with (
            tc.tile_pool(name="consts", bufs=1) as consts,
            tc.tile_pool(name="chain", bufs=1) as chain,
            tc.tile_pool(name="stream", bufs=8) as stream,
            tc.tile_pool(name="stream2", bufs=3) as stream2,
            tc.tile_pool(name="psT", bufs=1, space="PSUM") as psT,
            tc.tile_pool(name="psA", bufs=5, space="PSUM") as psA,
            tc.tile_pool(name="psB", bufs=2, space="PSUM") as psB,
        ):
            # ---- small loads first (sync ring): xrep/corep/blobs ----
            S = b_loc // 128
            b42_f = consts.tile([42, 832], F32)
            nc.sync.dma_start(b42_f[:], b42_d[:])
            b128_f = consts.tile([128, 524], F32)
            nc.sync.dma_start(b128_f[:], b128_d[:])
            xrep = chain.tile([42, b_loc], BF16, tag="bigA")
            corep = chain.tile([42, b_loc], BF16, tag="bigB")
            hb = b_loc // 2
            nc.sync.dma_start(xrep[:, 0:hb], xr_d[:, 0:hb])
            nc.sync.dma_start(corep[:, 0:hb], cor_d[:, 0:hb])
            nc.sync.dma_start(xrep[:, hb:], xr_d[:, hb:])
            corep_dma = nc.sync.dma_start(corep[:, hb:], cor_d[:, hb:])
            b42 = consts.tile([42, 832], BF16)
            nc.vector.tensor_copy(b42[:], b42_f[:])
            b128 = consts.tile([128, 524], BF16)
            nc.vector.tensor_copy(b128[:], b128_f[:])
            W0f = b42[:, 0:64]
            S64 = b42[:, 64:704]
            B1 = b42[:, 704:768]
            B2 = b42[:, 768:800]
            S3 = b42[:, 800:832]
            W1s = b128[:, 0:320]
            W2lo = b128[:, 320:352]
            W2hi = b128[:, 352:384]
            R3 = b128[:, 384:396]
            ident_b = b128[:, 396:524]
            DT = 2048
            cbs = []
            for p in range(5):
                cb_t = chain.tile([128, b_loc], BF16, tag=f"cb{p}")
                cbs.append(cb_t)
            # ---- u0coT = xrep * corep (feature-major, split for overlap) ----
            u0coT = chain.tile([42, b_loc], BF16)
            for qq in range(4):
                sl = slice(qq * (b_loc // 4), (qq + 1) * (b_loc // 4))
                nc.vector.tensor_tensor(
                    out=u0coT[:, sl], in0=xrep[:, sl], in1=corep[:, sl],
                    op=mybir.AluOpType.mult,
                )

            # ---- L0: z1T = W0f^T @ u0coT ; relu -> x1dup rows 0:64 ----
            x1dup = chain.tile([128, b_loc], BF16)
            x2dual = chain.tile([128, b_loc // 2], BF16)
            for c in range(CHUNKS):
                pz = psA.tile([H, 512], F32, tag="z")
                nc.tensor.matmul(pz[:], W0f[:], u0coT[:, c * 512:(c + 1) * 512])
                nc.scalar.activation(
                    x1dup[:H, c * 512:(c + 1) * 512], pz[:],
                    mybir.ActivationFunctionType.Relu,
                )
            # duplicate rows 0:64 -> 64:128 (sbuf->sbuf DMA partition move)
            for qq in range(4):
                sl = slice(qq * (b_loc // 4), (qq + 1) * (b_loc // 4))
                nc.sync.dma_start(x1dup[H:, sl], x1dup[:H, sl])
            # cbrep loads: sync ring, behind x1dup so the input pipeline and
            # relus are never starved; slices arrive just-in-time for z2
            for c4 in range(b_loc // DT):
                slc = slice(c4 * DT, (c4 + 1) * DT)
                for p in range(5):
                    nc.sync.dma_start(cbs[p][:, slc], cb_d[p, :, slc])

            # ---- L1: z2 = sum_p W1s_p^T @ (x1dup * cb_p) + B1^T @ u0coT ----
            # col-tiled: chunk pair (2d, 2d+1) -> one [128, 512] psum bank
            for c4 in range(CHUNKS // 4):
                xps = []
                for p in range(5):
                    xp = stream.tile([128, DT], BF16, tag="xp")
                    nc.vector.tensor_tensor(
                        out=xp[:], in0=x1dup[:, c4 * DT:(c4 + 1) * DT],
                        in1=cbs[p][:, c4 * DT:(c4 + 1) * DT],
                        op=mybir.AluOpType.mult,
                    )
                    xps.append(xp)
                for dd in range(2):  # two chunk-pairs per TT block
                    pz2 = psA.tile([128, 512], F32, tag="z")
                    for p in range(5):
                        for h in range(2):
                            cc = 2 * dd + h
                            nc.tensor.matmul(
                                pz2[64 * h:64 * h + 64, :],
                                W1s[:, p * H:(p + 1) * H],
                                xps[p][:, cc * 512:(cc + 1) * 512],
                                tile_position=(0, 64 * h),
                                start=(p == 0), stop=False,
                                skip_group_check=True,
                            )
                    for h in range(2):
                        c = 4 * c4 + 2 * dd + h
                        nc.tensor.matmul(
                            pz2[64 * h:64 * h + 64, :], B1[:],
                            u0coT[:, c * 512:(c + 1) * 512],
                            tile_position=(0, 64 * h),
                            start=False, stop=True,
                            skip_group_check=True,
                        )
                    d_abs = 2 * c4 + dd
                    nc.scalar.activation(
                        x2dual[:, d_abs * 512:(d_abs + 1) * 512], pz2[:],
                        mybir.ActivationFunctionType.Relu,
                    )

            # ---- L2 (4-chunk groups, partition-stacked; t-scheme, 30-wide) ----
            y_bm = chain.tile([128, S * CO], F32)
            for g in range(GROUPS):
                pt2 = psB.tile([128, 512], F32, tag="cbps")
                pcb3 = psB.tile([128, 512], F32, tag="cbps")
                for q in range(4):
                    c = 4 * g + q
                    d_abs, h = divmod(c, 2)
                    W2v = W2lo if h == 0 else W2hi
                    nc.tensor.matmul(
                        pt2[32 * q:32 * q + 32, :], W2v[:],
                        x2dual[:, d_abs * 512:(d_abs + 1) * 512],
                        tile_position=(0, 32 * q),
                        start=True, stop=False,
                        skip_group_check=True,
                    )
                for q in range(4):
                    c = 4 * g + q
                    nc.tensor.matmul(
                        pt2[32 * q:32 * q + 32, :], B2[:],
                        u0coT[:, c * 512:(c + 1) * 512],
                        tile_position=(0, 32 * q),
                        start=False, stop=True,
                        skip_group_check=True,
                    )
                for q in range(4):
                    c = 4 * g + q
                    nc.tensor.matmul(
                        pcb3[32 * q:32 * q + 32, :], S3[:],
                        u0coT[:, c * 512:(c + 1) * 512],
                        tile_position=(0, 32 * q),
                    )
                t2_sb = stream2.tile([128, 512], BF16, tag="t2_sb")
                nc.vector.tensor_copy(t2_sb[:], pt2[:])
                cb3_sb = stream2.tile([128, 512], BF16, tag="cb3_sb")
                nc.scalar.activation(
                    cb3_sb[:], pcb3[:], mybir.ActivationFunctionType.Copy
                )
                m2_sb = stream2.tile([128, 512], BF16, tag="m2_sb")
                nc.vector.tensor_tensor(
                    out=m2_sb[:], in0=t2_sb[:], in1=cb3_sb[:],
                    op=mybir.AluOpType.mult,
                )
                pyT = psA.tile([12, 512], F32, tag="z")
                nc.tensor.matmul(pyT[:], R3[:], m2_sb[:])
                yT_sb = stream2.tile([12, 512], BF16, tag="yT_sb")
                nc.vector.tensor_copy(yT_sb[:], pyT[:])
                for j in range(4):
                    pyb = psT.tile([128, 12], BF16, tag="tp_in")
                    nc.tensor.transpose(
                        pyb[:], yT_sb[:, j * 128:(j + 1) * 128],
                        ident_b[:12, :12],
                    )
                    y5 = y_bm[:].rearrange(
                        "p (gg q j o) -> p gg q j o", q=4, j=4, o=CO
                    )
                    nc.vector.tensor_copy(
                        y5[:, g, :, j, :],
                        pyb[:].rearrange("p (q o) -> p q o", o=CO),
                    )

            nc.sync.dma_start(
                out_d[:].rearrange("(p s) o -> p (s o)", p=128), y_bm[:]
            )
    nc.compile()
    return nc


_NC_CACHE = {}


def get_nc(b_loc=B_LOC):
    if b_loc not in _NC_CACHE:
        nc = bacc.Bacc(None, target_bir_lowering=False)
        _NC_CACHE[b_loc] = build(nc, b_loc)
    return _NC_CACHE[b_loc]


def kernel(input, co_mat, W0, W1, W2, b0, b1, b2, _trace=False):
    input = np.asarray(input, np.float32)
    co_mat = np.asarray(co_mat, np.float32)
    consts = host_constants(
        np.asarray(W0, np.float32), np.asarray(W1, np.float32),
        np.asarray(W2, np.float32), np.asarray(b0, np.float32),
        np.asarray(b1, np.float32), np.asarray(b2, np.float32),
    )
    nc = get_nc()
    in_maps = []
    for k in range(N_CORES):
        sl = slice(k * B_LOC, (k + 1) * B_LOC)
        xr, cr = make_reps(input[sl], co_mat[sl])
        m = {"xrep": xr, "corep": cr, "cbrep": make_cbrep(co_mat[sl])}
        m.update(consts)
        in_maps.append(m)
    res = run_bass_kernel_spmd(
        nc, in_maps, core_ids=list(range(N_CORES)), trace=_trace
    )
    out = np.concatenate([res.results[k]["out"] for k in range(N_CORES)], axis=0)
    if _trace:
        kernel.last_exec_time_ns = res.exec_time_ns
    return out


kernel.last_exec_time_ns = None
